# revision 5
# baseline (speedup 1.0000x reference)
"""Multi-head attention (S=2048, B=4, H=1024, NH=16) on 8 Trainium2 NeuronCores.

Sharding: each core handles 2 batches x 4 heads (batch pairs balanced by
valid length; tensor-parallel over heads). Within a core everything is bf16
matmul / fp32 accumulate.

Since the raw q/k/v inputs are zeroed at padded positions, projected k and v
are exactly zero there, so scores for padded keys are exactly 0 and
exp(0)=1: no mask bias is needed; each padded key contributes exactly 1.0
to Z (subtracted via the per-slot npad input) and nothing to P@V.

Per (qc, p, kc) the PE work is three ~one-stream slots:
  - scores: 4 quadrant-tiled MMs (2 heads x 2 k-halves), two parallel
    column-group streams
  - PV pair (h0 || h1 col-tiled), Z pair (h0 || h1 col-tiled)
exp runs on ScalarE with scale only (no bias); in the slot-1 attention
phase exp is fused over kc pairs (3 rotating PSUM score slots); in the
slot-0 phase it is per-kc (2 slots) and slot-1's projections + input DMAs
are interleaved into the emission stream to fill PE slack. Wo PSUM lives
in bank-aligned slices of the score buffer. The last q-chunk is pruned to
the valid width (rounded to 64). Normalization: rz = recip(Z - npad);
ab = attn * rz; padded-query columns are zeroed post-Wo (ysb = yps * kqr).
"""
import sys

if "/opt/trn_rl_repo" not in sys.path:
    sys.path.insert(0, "/opt/trn_rl_repo")

import math
from itertools import permutations

import ml_dtypes
import numpy as np

import concourse.bass as bass
import concourse.mybir as mybir
import concourse.tile as tile
from concourse import bacc
from concourse.bass_utils import run_bass_kernel_spmd

S, B, H, NH, DK = 2048, 4, 1024, 16, 64
N_CORES = 8
BF16 = mybir.dt.bfloat16
F32 = mybir.dt.float32
NPBF16 = ml_dtypes.bfloat16
SCALE = 1.0 / math.sqrt(DK)

_prog_cache: dict = {}


def _build_program(NQ, NK, W):
    """One SPMD program. Per slot s: NQ[s] 512-wide q chunks (last pruned to
    W[s]), NK[s] 128-wide k chunks. Slot 0 should be the smaller workload."""
    NSCK = [(nk * 128 + 511) // 512 for nk in NK]
    KW = [nk * 128 - (nsc - 1) * 512 for nk, nsc in zip(NK, NSCK)]  # last k-sc width
    QW = [(nq - 1) * 512 + w for nq, w in zip(NQ, W)]  # loaded q extent
    nc = bacc.Bacc("TRN2", target_bir_lowering=False, debug=False,
                   num_devices=N_CORES)

    d_in = {}
    for s in range(2):
        d_in[f"qT{s}"] = nc.dram_tensor(f"qT{s}", [H, S], BF16, kind="ExternalInput")
        d_in[f"kT{s}"] = nc.dram_tensor(f"kT{s}", [H, S], BF16, kind="ExternalInput")
        d_in[f"vT{s}"] = nc.dram_tensor(f"vT{s}", [H, S], BF16, kind="ExternalInput")
        d_in[f"kq{s}"] = nc.dram_tensor(f"kq{s}", [4, 512], F32, kind="ExternalInput")
        d_in[f"pd{s}"] = nc.dram_tensor(f"pd{s}", [1, 1], F32, kind="ExternalInput")
    d_in["wqT"] = nc.dram_tensor("wqT", [H, 256], BF16, kind="ExternalInput")
    d_in["wkT"] = nc.dram_tensor("wkT", [H, 256], BF16, kind="ExternalInput")
    d_in["wvT"] = nc.dram_tensor("wvT", [H, 256], BF16, kind="ExternalInput")
    d_in["woT"] = nc.dram_tensor("woT", [256, H], BF16, kind="ExternalInput")
    d_out = [nc.dram_tensor(f"y{s}", [H, S], BF16, kind="ExternalOutput")
             for s in range(2)]

    with tile.TileContext(nc) as tc:
        with tc.tile_pool(name="wpool", bufs=1) as wpool, \
             tc.tile_pool(name="inp", bufs=3) as inp, \
             tc.tile_pool(name="in8", bufs=1) as in8, \
             tc.tile_pool(name="persist", bufs=1) as persist, \
             tc.tile_pool(name="probs", bufs=3) as probsp, \
             tc.tile_pool(name="small", bufs=4) as small, \
             tc.tile_pool(name="att", bufs=3) as attp, \
             tc.tile_pool(name="yst", bufs=3) as ystp:

            # --- constants / weights ---
            wq = [wpool.tile([128, 256], BF16, name=f"wq{i}", tag=f"wq{i}")
                  for i in range(8)]
            wk = [wpool.tile([128, 256], BF16, name=f"wk{i}", tag=f"wk{i}")
                  for i in range(8)]
            wv = [wpool.tile([128, 256], BF16, name=f"wv{i}", tag=f"wv{i}")
                  for i in range(8)]
            wo = [wpool.tile([128, H], BF16, name=f"wo{j}", tag=f"wo{j}")
                  for j in range(2)]
            for i in range(8):
                nc.sync.dma_start(out=wq[i][:], in_=d_in["wqT"].ap()[i * 128:(i + 1) * 128, :])
                nc.sync.dma_start(out=wk[i][:], in_=d_in["wkT"].ap()[i * 128:(i + 1) * 128, :])
                nc.sync.dma_start(out=wv[i][:], in_=d_in["wvT"].ap()[i * 128:(i + 1) * 128, :])
            for j in range(2):
                nc.sync.dma_start(out=wo[j][:], in_=d_in["woT"].ap()[j * 128:(j + 1) * 128, :])
            ones = wpool.tile([128, 64], BF16, name="ones", tag="ones")
            nc.vector.memset(ones[:], 1.0)
            npadt = [wpool.tile([128, 1], F32, name=f"npad{s}", tag=f"npad{s}")
                     for s in range(2)]
            for s in range(2):
                nc.gpsimd.dma_start(
                    out=npadt[s][:],
                    in_=bass.AP(tensor=d_in[f"pd{s}"], offset=0,
                                ap=[[0, 128], [1, 1]]))

            # --- persistent projection outputs ---
            qTp = [[persist.tile([128, NQ[s] * 512], BF16, name=f"qTp{s}_{p}",
                                 tag=f"qTp{s}_{p}")
                    for p in range(2)] for s in range(2)]
            kTp = [[persist.tile([128, NSCK[s] * 512], BF16, name=f"kTp{s}_{p}",
                                 tag=f"kTp{s}_{p}")
                    for p in range(2)] for s in range(2)]
            vp = [[persist.tile([128, 256], BF16, name=f"vp{s}_{st}", tag=f"vp{s}_{st}")
                   for st in range(NK[s])] for s in range(2)]

            def q_sc_width(s, sc):
                return 512 if sc < NQ[s] - 1 else W[s]

            def k_sc_width(s, sc):
                return 512 if sc < NSCK[s] - 1 else KW[s]

            def emit_proj_streamed(s, pool):
                """ic-outer projections with streamed inputs (slot 0 head)."""
                for kind, wts, dname, nsc, wfn, outtiles in (
                        ("q", wq, f"qT{s}", NQ[s], lambda sc: q_sc_width(s, sc), qTp[s]),
                        ("k", wk, f"kT{s}", NSCK[s], lambda sc: k_sc_width(s, sc), kTp[s])):
                    ext = (nsc - 1) * 512 + wfn(nsc - 1)
                    ps = [[pool.tile([128, 512], F32,
                                     name=f"pj{kind}{s}_{ft}_{sc}",
                                     tag=f"pj_{ft}_{sc}")
                           for sc in range(nsc)] for ft in range(2)]
                    for ic in range(8):
                        it = inp.tile([128, nsc * 512], BF16,
                                      name=f"in{kind}{s}_{ic}", tag="inp")
                        nc.sync.dma_start(
                            out=it[:, 0:ext],
                            in_=d_in[dname].ap()[ic * 128:(ic + 1) * 128, 0:ext])
                        for ft in range(2):
                            for sc in range(nsc):
                                cw = wfn(sc)
                                nc.tensor.matmul(
                                    out=ps[ft][sc][:, 0:cw],
                                    lhsT=wts[ic][:, ft * 128:(ft + 1) * 128],
                                    rhs=it[:, sc * 512:sc * 512 + cw],
                                    start=(ic == 0), stop=(ic == 7))
                    for ft in range(2):
                        for sc in range(nsc):
                            cw = wfn(sc)
                            if kind == "q":
                                nc.vector.tensor_copy(
                                    outtiles[ft][:, sc * 512:sc * 512 + cw],
                                    ps[ft][sc][:, 0:cw])
                            else:
                                nc.scalar.copy(
                                    outtiles[ft][:, sc * 512:sc * 512 + cw],
                                    ps[ft][sc][:, 0:cw])
                for st0 in range(0, NK[s], 8):
                    sts = range(st0, min(st0 + 8, NK[s]))
                    psv = {st: pool.tile([128, 256], F32, name=f"pjv{s}_{st}",
                                         tag=f"pj_{(st - st0) // 4}_{(st - st0) % 4}")
                           for st in sts}
                    for ic in range(8):
                        it = inp.tile([128, 1024], BF16,
                                      name=f"inv{s}_{st0}_{ic}", tag="inp")
                        nc.sync.dma_start(
                            out=it[:, 0:len(sts) * 128],
                            in_=d_in[f"vT{s}"].ap()[ic * 128:(ic + 1) * 128,
                                                    st0 * 128:(st0 + len(sts)) * 128])
                        for st in sts:
                            nc.tensor.matmul(
                                out=psv[st][:],
                                lhsT=it[:, (st - st0) * 128:(st - st0 + 1) * 128],
                                rhs=wv[ic][:, :],
                                start=(ic == 0), stop=(ic == 7))
                    for st in sts:
                        if st % 2:
                            nc.scalar.copy(vp[s][st][:], psv[st][:])
                        else:
                            nc.vector.tensor_copy(vp[s][st][:], psv[st][:])

            def proj1_feeder(s, pool):
                """Generator: yields after each small chunk of slot-s
                prefetch-DMA / projection work. pool provides 1 PSUM bank."""
                tiles = {}
                for kind, dname, w in (("v", f"vT{s}", NK[s] * 128),
                                       ("q", f"qT{s}", QW[s]),
                                       ("k", f"kT{s}", NK[s] * 128)):
                    for ic in range(8):
                        it = in8.tile([128, w], BF16, name=f"pf{kind}{s}_{ic}",
                                      tag=f"pf{kind}{ic}")
                        nc.sync.dma_start(
                            out=it[:],
                            in_=d_in[dname].ap()[ic * 128:(ic + 1) * 128, 0:w])
                        tiles[(kind, ic)] = it
                        yield
                for st in range(NK[s]):
                    pj = pool.tile([128, 512], F32, name=f"rpjv{s}_{st}", tag="pj1")
                    for ic in range(0, 8, 2):
                        for i2 in (ic, ic + 1):
                            nc.tensor.matmul(
                                out=pj[:, 0:256],
                                lhsT=tiles[("v", i2)][:, st * 128:(st + 1) * 128],
                                rhs=wv[i2][:, :],
                                start=(i2 == 0), stop=(i2 == 7))
                        yield
                    if st % 2:
                        nc.scalar.copy(vp[s][st][:], pj[:, 0:256])
                    else:
                        nc.vector.tensor_copy(vp[s][st][:], pj[:, 0:256])
                    yield
                for kind, wts, nsc, wfn, outtiles in (
                        ("q", wq, NQ[s], lambda sc: q_sc_width(s, sc), qTp[s]),
                        ("k", wk, NSCK[s], lambda sc: k_sc_width(s, sc), kTp[s])):
                    for ft in range(2):
                        for sc in range(nsc):
                            cw = wfn(sc)
                            pj = pool.tile([128, 512], F32,
                                           name=f"rpj{kind}{s}_{ft}_{sc}", tag="pj1")
                            for ic in range(0, 8, 2):
                                for i2 in (ic, ic + 1):
                                    nc.tensor.matmul(
                                        out=pj[:, 0:cw],
                                        lhsT=wts[i2][:, ft * 128:(ft + 1) * 128],
                                        rhs=tiles[(kind, i2)][:, sc * 512:sc * 512 + cw],
                                        start=(i2 == 0), stop=(i2 == 7))
                                yield
                            nc.vector.tensor_copy(
                                outtiles[ft][:, sc * 512:sc * 512 + cw],
                                pj[:, 0:cw])
                            yield

            def emit_attention(s, sc_big, nslot, fuse, pat, pz, feeder=None):
                """sc_big: [128, nslot*1024] fp32 PSUM tile (rotating kc
                slots); Wo PSUM is carved from its last slot's banks."""
                sc4 = sc_big.tensor.reshape([128, nslot, 2, 512])

                def feed(n):
                    if feeder is not None:
                        for _ in range(n):
                            next(feeder, None)

                ybase = (nslot - 1) * 1024

                for qc in range(NQ[s]):
                    w = q_sc_width(s, qc)
                    kqr = small.tile([128, 512], F32, name=f"kqr{s}_{qc}",
                                     tag="kqr")
                    nc.gpsimd.dma_start(
                        out=kqr[:, 0:w],
                        in_=bass.AP(tensor=d_in[f"kq{s}"], offset=qc * 512,
                                    ap=[[0, 128], [1, w]]))
                    att_sb = []
                    for p in range(2):
                        attn = pat.tile([128, 512], F32,
                                        name=f"at{s}_{qc}_{p}", tag="at")
                        zps = pz.tile([128, 512], F32,
                                      name=f"z{s}_{qc}_{p}", tag="z")

                        def emit_scores(kc):
                            sl = (kc % nslot) * 1024
                            for kh in range(2):
                                for h in range(2):
                                    nc.tensor.matmul(
                                        out=sc_big[kh * 64:(kh + 1) * 64,
                                                   sl + h * 512:sl + h * 512 + w],
                                        lhsT=kTp[s][p][h * 64:(h + 1) * 64,
                                                       kc * 128 + kh * 64:kc * 128 + (kh + 1) * 64],
                                        rhs=qTp[s][p][h * 64:(h + 1) * 64,
                                                      qc * 512:qc * 512 + w],
                                        start=True, stop=True)

                        def emit_exp(kc0, nkc):
                            pr = probsp.tile([128, 2048], BF16,
                                             name=f"pr{s}_{qc}_{p}_{kc0}",
                                             tag="pr")
                            pr4 = pr.tensor.reshape([128, 2, 2, 512])
                            s0 = kc0 % nslot
                            if nkc == 2:
                                s1 = (kc0 + 1) % nslot
                                if s1 == s0 + 1:
                                    in_ap = sc4[:, s0:s0 + 2, :, 0:w]
                                else:
                                    in_ap = sc4[:, s0::s1 - s0, :, 0:w]
                                out_ap = pr4[:, 0:2, :, 0:w]
                            else:
                                in_ap = sc4[:, s0:s0 + 1, :, 0:w]
                                out_ap = pr4[:, 0:1, :, 0:w]
                            nc.scalar.activation(
                                out=out_ap, in_=in_ap,
                                func=mybir.ActivationFunctionType.Exp,
                                scale=SCALE)
                            return pr

                        def emit_pvz(kc, pr, pri):
                            first, last = kc == 0, kc == NK[s] - 1
                            for h in range(2):
                                nc.tensor.matmul(
                                    out=attn[h * 64:(h + 1) * 64, 0:w],
                                    lhsT=vp[s][kc][:, p * 128 + h * 64:p * 128 + (h + 1) * 64],
                                    rhs=pr[:, pri * 1024 + h * 512:pri * 1024 + h * 512 + w],
                                    start=first, stop=last)
                            for h in range(2):
                                nc.tensor.matmul(
                                    out=zps[h * 64:(h + 1) * 64, 0:w],
                                    lhsT=ones[:, :],
                                    rhs=pr[:, pri * 1024 + h * 512:pri * 1024 + h * 512 + w],
                                    start=first, stop=last)

                        if fuse:
                            emit_scores(0)
                            emit_scores(1)
                            pr_cur = emit_exp(0, 2)
                            for j in range(0, NK[s] - 1, 2):
                                pr = pr_cur
                                if j + 3 < NK[s]:
                                    emit_scores(j + 2)
                                    emit_scores(j + 3)
                                    pr_cur = emit_exp(j + 2, 2)
                                elif j + 2 < NK[s]:
                                    emit_scores(j + 2)
                                    pr_cur = emit_exp(j + 2, 1)
                                emit_pvz(j, pr, 0)
                                emit_pvz(j + 1, pr, 1)
                                feed(2)
                            if NK[s] % 2:
                                emit_pvz(NK[s] - 1, pr_cur, 0)
                        else:
                            emit_scores(0)
                            pr_cur = emit_exp(0, 1)
                            for kc in range(NK[s]):
                                pr = pr_cur
                                if kc + 1 < NK[s]:
                                    emit_scores(kc + 1)
                                    pr_cur = emit_exp(kc + 1, 1)
                                emit_pvz(kc, pr, 0)
                                feed(2)
                        # normalize: rz = recip(Z - npad); ab = attn * rz
                        zadj = small.tile([128, 512], F32,
                                          name=f"za{s}_{qc}_{p}", tag="za")
                        nc.vector.tensor_scalar_sub(
                            zadj[:, 0:w], zps[:, 0:w], npadt[s][:, 0:1])
                        rz = small.tile([128, 512], F32,
                                        name=f"rz{s}_{qc}_{p}", tag="rz")
                        nc.vector.reciprocal_approx_fast(out=rz[:, 0:w],
                                                         in_=zadj[:, 0:w])
                        ab = attp.tile([128, 512], BF16,
                                       name=f"ab{s}_{qc}_{p}", tag=f"ab{p}")
                        nc.vector.tensor_mul(ab[:, 0:w], attn[:, 0:w], rz[:, 0:w])
                        att_sb.append(ab)
                        feed(1)
                    # Wo: col-tiled pairs into score-slot banks; kqr post-Wo
                    for ot in range(8):
                        yoff = ybase + (ot % 2) * 512
                        yps = sc_big[:, yoff:yoff + 512]
                        for j in range(2):
                            for h in range(2):
                                nc.tensor.matmul(
                                    out=sc_big[h * 64:(h + 1) * 64, yoff:yoff + w],
                                    lhsT=wo[j][:, ot * 128 + h * 64:ot * 128 + (h + 1) * 64],
                                    rhs=att_sb[j][:, 0:w],
                                    start=(j == 0), stop=(j == 1),
                                    skip_group_check=True)
                        ysb = ystp.tile([128, 512], BF16,
                                        name=f"ysb{s}_{qc}_{ot}", tag="ysb")
                        nc.vector.tensor_mul(ysb[:, 0:w],
                                             sc_big[:, yoff:yoff + w],
                                             kqr[:, 0:w])
                        nc.gpsimd.dma_start(
                            out=d_out[s].ap()[ot * 128:(ot + 1) * 128,
                                              qc * 512:qc * 512 + w],
                            in_=ysb[:, 0:w])
                        feed(1)

            # phase A: slot 0 projections use the full PSUM (released after)
            with tc.tile_pool(name="pproj", bufs=1, space="PSUM") as pproj:
                emit_proj_streamed(0, pproj)
            # phase B: slot-0 attention (2 slots, unfused exp) + slot-1
            # prefetch/proj interleaved; 4 + 2 + 1 + 1 = 8 banks
            with tc.tile_pool(name="pscB", bufs=1, space="PSUM") as pscB, \
                 tc.tile_pool(name="patB", bufs=2, space="PSUM") as patB, \
                 tc.tile_pool(name="pzB", bufs=1, space="PSUM") as pzB, \
                 tc.tile_pool(name="ppj1", bufs=1, space="PSUM") as ppj1:
                scB = pscB.tile([128, 2048], F32, name="scB", tag="scB")
                feeder = proj1_feeder(1, ppj1)
                emit_attention(0, scB, 2, False, patB, pzB, feeder)
                for _ in feeder:  # drain remaining slot-1 proj work
                    pass
            # phase C: slot-1 attention (3 slots, fused exp); 6 + 1 + 1 = 8
            with tc.tile_pool(name="pscC", bufs=1, space="PSUM") as pscC, \
                 tc.tile_pool(name="patC", bufs=1, space="PSUM") as patC, \
                 tc.tile_pool(name="pzC", bufs=1, space="PSUM") as pzC:
                scC = pscC.tile([128, 3072], F32, name="scC", tag="scC")
                emit_attention(1, scC, 3, True, patC, pzC)
    nc.compile()
    return nc


def _get_program(NQ, NK, W):
    key = (tuple(NQ), tuple(NK), tuple(W))
    if key not in _prog_cache:
        _prog_cache[key] = _build_program(list(NQ), list(NK), list(W))
    return _prog_cache[key]


def kernel(value, key, query, padding_mask, Wq, Wk, Wv, Wo):
    value = np.asarray(value)
    key = np.asarray(key)
    query = np.asarray(query)
    padding_mask = np.asarray(padding_mask)
    Wq, Wk, Wv, Wo = (np.asarray(a) for a in (Wq, Wk, Wv, Wo))

    lengths = (~padding_mask).sum(axis=0).astype(int)  # (B,)

    # --- batch pairing: assign batches to (group, slot) minimizing baked work ---
    def slot_counts(assign):
        lm = [max(int(lengths[assign[g][sl]]) for g in range(2))
              for sl in range(2)]
        nq = [(l + 511) // 512 for l in lm]
        nk = [(l + 127) // 128 for l in lm]
        w = [min(512, ((l - (q - 1) * 512 + 63) // 64) * 64)
             for l, q in zip(lm, nq)]
        return nq, nk, w

    best = None
    for perm in permutations(range(B)):
        a = ((perm[0], perm[1]), (perm[2], perm[3]))
        nq, nk, w = slot_counts(a)
        c = sum(k * 128 * ((q - 1) * 512 + ww) for q, k, ww in zip(nq, nk, w))
        if best is None or c < best[0]:
            best = (c, a)
    assign = best[1]
    nq, nk, w = slot_counts(assign)
    # slot 0 should be the smaller workload
    if nq[0] * nk[0] > nq[1] * nk[1]:
        assign = tuple((g[1], g[0]) for g in assign)
        nq, nk, w = slot_counts(assign)
    NQ, NK, W = nq, nk, w

    nc = _get_program(NQ, NK, W)

    # --- per-core inputs ---
    WqT = np.ascontiguousarray(Wq.T).astype(NPBF16)
    WkT = np.ascontiguousarray(Wk.T).astype(NPBF16)
    WvT = np.ascontiguousarray(Wv.T).astype(NPBF16)
    WoT = np.ascontiguousarray(Wo.T).astype(NPBF16)

    batch_qT, batch_kT, batch_vT, batch_kq = {}, {}, {}, {}
    for b in range(B):
        batch_qT[b] = np.ascontiguousarray(query[:, b, :].T).astype(NPBF16)
        batch_kT[b] = np.ascontiguousarray(key[:, b, :].T).astype(NPBF16)
        batch_vT[b] = np.ascontiguousarray(value[:, b, :].T).astype(NPBF16)
        batch_kq[b] = (np.arange(S).reshape(4, 512) < lengths[b]).astype(np.float32)

    in_maps = []
    for c in range(N_CORES):
        g, hq = c // 4, c % 4
        f0 = hq * 256
        m = {
            "wqT": np.ascontiguousarray(WqT[:, f0:f0 + 256]),
            "wkT": np.ascontiguousarray(WkT[:, f0:f0 + 256]),
            "wvT": np.ascontiguousarray(WvT[:, f0:f0 + 256]),
            "woT": np.ascontiguousarray(WoT[f0:f0 + 256, :]),
        }
        for sl in range(2):
            b = assign[g][sl]
            m[f"qT{sl}"] = batch_qT[b]
            m[f"kT{sl}"] = batch_kT[b]
            m[f"vT{sl}"] = batch_vT[b]
            m[f"kq{sl}"] = batch_kq[b]
            m[f"pd{sl}"] = np.full((1, 1), NK[sl] * 128 - int(lengths[b]),
                                   dtype=np.float32)
        in_maps.append(m)

    res = run_bass_kernel_spmd(nc, in_maps, list(range(N_CORES)))

    # --- gather: sum 4 head-quad partials per batch, transpose ---
    out = np.zeros((S, B, H), dtype=np.float32)
    for g in range(2):
        for sl in range(2):
            b = assign[g][sl]
            acc = np.zeros((H, S), dtype=np.float32)
            for hq in range(4):
                c = g * 4 + hq
                acc += res.results[c][f"y{sl}"].astype(np.float32)
            out[:, b, :] = acc.T
    return out
